# revision 21
# baseline (speedup 1.0000x reference)
"""Trainium2 Bass kernel for the KAN autonomous ODE func:
    s   = tanh(h[:, :, None] * alpha + beta)            # [B, H, K]
    phi = einsum("bik,oik->bo", s, W) / K               # [B, O]
    out = tanh(phi) * gain + bias                       # [B, O]
with B=2048, H=1024, K=16, O=H.

Sharding (8 cores): 4 batch shards x 2 output shards. Each core computes
out[bshard, oshard] as a [O_SH=512, B_SH=512] tile via a bf16 GEMM with
contraction dim H*K=16384 accumulated in fp32 PSUM. The basis expansion s
is built on-chip by the scalar engine (tanh(alpha_k*h + beta_k) is exactly
ACT's fused func(scale*x+bias)). The 1/K scale is folded into W on the host
(power of two -> exact). No collectives; the host slices inputs and
reassembles the output.
"""

import sys

import numpy as np

if "/opt/trn_rl_repo" not in sys.path:
    sys.path.insert(0, "/opt/trn_rl_repo")

import ml_dtypes

import concourse.bass as bass
import concourse.tile as tile
from concourse import bacc, mybir
from concourse.bass_utils import run_bass_kernel_spmd

B, H, K = 2048, 1024, 16
RB, CO = 4, 2                      # batch shards x output shards
B_SH = B // RB                     # 512 batch rows per core
O_SH = H // CO                     # 512 output cols per core
NCH = 8                            # i-chunks of 128 within H
OT = O_SH // 128                   # 4 psum output tiles per core

F32 = mybir.dt.float32
BF16 = mybir.dt.bfloat16

_CACHE = {}


def _build():
    """Build + compile the Tile kernel once per process."""
    if "nc" in _CACHE:
        return _CACHE["nc"]

    nc = bacc.Bacc(
        "TRN2",
        target_bir_lowering=False,
        debug=False,
        enable_asserts=False,
        num_devices=RB * CO,
    )

    hT = nc.dram_tensor("hT", [128, NCH, B_SH], BF16, kind="ExternalInput").ap()
    wT = nc.dram_tensor("wT", [K, 128, NCH, O_SH], BF16, kind="ExternalInput").ap()
    ab = nc.dram_tensor("ab", [128, 2 * K], F32, kind="ExternalInput").ap()
    gb = nc.dram_tensor("gb", [128, 2 * OT], F32, kind="ExternalInput").ap()
    out = nc.dram_tensor("out", [OT, 128, B_SH], F32, kind="ExternalOutput").ap()

    HCH = NCH // 2  # half of the i-chunks, for split h/s pipelining

    with tile.TileContext(nc) as tc:
        with (
            tc.tile_pool(name="const", bufs=1) as const_pool,
            tc.tile_pool(name="h", bufs=1) as h_pool,
            tc.tile_pool(name="w", bufs=3) as w_pool,
            tc.tile_pool(name="s", bufs=4) as s_pool,
            tc.tile_pool(name="o", bufs=2) as o_pool,
            tc.tile_pool(name="psum", bufs=1, space=bass.MemorySpace.PSUM) as psum_pool,
        ):
            ab_t = const_pool.tile([128, 2 * K], F32, tag="ab")
            nc.sync.dma_start(ab_t[:], ab[:])
            gb_t = const_pool.tile([128, 2 * OT], F32, tag="gb")
            nc.sync.dma_start(gb_t[:], gb[:])

            # PE pre-warm: dummy accumulations into a scratch PSUM bank while
            # the initial h/W DMAs are in flight, so the HAM clock gate is at
            # K=8/8 (2.4 GHz) when the real matmuls start.
            warm_sb = const_pool.tile([128, 128], F32, tag="warm")
            nc.vector.memset(warm_sb[:], 0.0)
            warm_ps = psum_pool.tile([128, 128], F32, tag="warmps")
            N_WARM = 24
            for i in range(N_WARM):
                nc.tensor.matmul(
                    warm_ps[:],
                    warm_sb[:],
                    warm_sb[:],
                    start=(i == 0),
                    stop=(i == N_WARM - 1),
                )

            # h in two half tiles on two queues so the first ACT only waits
            # for the first half. First half on the scalar queue (feeds the
            # first ACT); second half on sync, emitted behind w0's b-half.
            h_ta = h_pool.tile([128, HCH, B_SH], BF16, tag="ha", name="h_ta")
            h_tb = h_pool.tile([128, HCH, B_SH], BF16, tag="hb", name="h_tb")
            nc.scalar.dma_start(h_ta[:], hT[:, :HCH, :])

            # One PSUM tile per output bank so each bank's epilogue can
            # overlap the remaining banks' matmuls (deps are per-tile).
            psum_b = [
                psum_pool.tile([128, B_SH], F32, tag=f"acc{ot}", name=f"acc{ot}")
                for ot in range(OT)
            ]

            for k in range(K):
                # Each W slab is split across the two fast HWDGE queues
                # (scalar + sync); separate tiles so the first half's
                # matmuls don't wait on the second half's DMA.
                w_ka = w_pool.tile(
                    [128, HCH, O_SH], BF16, tag="wka", name=f"wka_{k}"
                )
                w_kb = w_pool.tile(
                    [128, HCH, O_SH], BF16, tag="wkb", name=f"wkb_{k}"
                )
                # Scalar issues only the first few a-half triggers (so its
                # first ACT isn't pushed back by trigger instructions);
                # gpsimd's SWDGE queue takes the rest mid-stream.
                eng_a = nc.scalar if k < 3 else nc.gpsimd
                eng_b = nc.sync if k % 2 == 0 else nc.gpsimd
                eng_a.dma_start(w_ka[:], wT[k, :, :HCH, :])
                eng_b.dma_start(w_kb[:], wT[k, :, HCH:, :])
                if k == 0:
                    nc.sync.dma_start(h_tb[:], hT[:, HCH:, :])

                # s in two halves so matmuls can start after half the tanh.
                s_k = [
                    s_pool.tile(
                        [128, HCH, B_SH], BF16, tag=f"sk{half}", name=f"sk{half}_{k}"
                    )
                    for half in range(2)
                ]
                for half, h_half in enumerate((h_ta, h_tb)):
                    nc.scalar.activation(
                        s_k[half][:],
                        h_half[:],
                        mybir.ActivationFunctionType.Tanh,
                        bias=ab_t[:, K + k : K + k + 1],
                        scale=ab_t[:, k : k + 1],
                    )

                def mm(c, ot):
                    w_half = w_ka if c < HCH else w_kb
                    nc.tensor.matmul(
                        psum_b[ot][:],
                        w_half[:, c % HCH, ot * 128 : (ot + 1) * 128],
                        s_k[c // HCH][:, c % HCH, :],
                        start=(k == 0 and c == 0),
                        stop=(k == K - 1 and c == NCH - 1),
                    )

                if k < K - 1:
                    for c in range(NCH):
                        for ot in range(OT):
                            mm(c, ot)
                else:
                    # Last k: finish PSUM banks one at a time so each bank's
                    # epilogue overlaps the remaining matmuls.
                    for ot in range(OT):
                        for c in range(NCH):
                            mm(c, ot)
                        o_t = o_pool.tile([128, B_SH], F32, tag="ot")
                        nc.scalar.activation(
                            o_t[:],
                            psum_b[ot][:],
                            mybir.ActivationFunctionType.Tanh,
                        )
                        o2_t = o_pool.tile([128, B_SH], F32, tag="o2")
                        nc.vector.tensor_scalar(
                            o2_t[:],
                            o_t[:],
                            gb_t[:, ot : ot + 1],
                            gb_t[:, OT + ot : OT + ot + 1],
                            mybir.AluOpType.mult,
                            mybir.AluOpType.add,
                        )
                        nc.sync.dma_start(out[ot], o2_t[:])

    nc.compile()
    _CACHE["nc"] = nc
    return nc


def _prep_inputs(h, W, alpha, beta, gain, bias):
    """Host-side slicing/layout. Returns in_maps for the 8 cores."""
    h = np.asarray(h, np.float32)
    W = np.asarray(W, np.float32)
    alpha = np.asarray(alpha, np.float32)
    beta = np.asarray(beta, np.float32)
    gain = np.asarray(gain, np.float32)
    bias = np.asarray(bias, np.float32)

    # W[o,i,k] -> wT[k, p, c, o] with i = c*128 + p; scale by 1/K (exact).
    Wr = np.transpose(W * (1.0 / K), (2, 1, 0))            # [K, H, O]
    Wr = Wr.reshape(K, NCH, 128, H).transpose(0, 2, 1, 3)  # [K, 128, NCH, O]
    Wr = np.ascontiguousarray(Wr).astype(ml_dtypes.bfloat16)

    ab = np.tile(np.concatenate([alpha, beta])[None, :], (128, 1)).astype(np.float32)
    ab = np.ascontiguousarray(ab)

    in_maps = []
    for rb in range(RB):
        h_sh = h[rb * B_SH : (rb + 1) * B_SH, :]            # [B_SH, H]
        hT = np.ascontiguousarray(
            h_sh.T.reshape(NCH, 128, B_SH).transpose(1, 0, 2)
        ).astype(ml_dtypes.bfloat16)                        # [128, NCH, B_SH]
        for co in range(CO):
            osl = slice(co * O_SH, (co + 1) * O_SH)
            w_core = np.ascontiguousarray(Wr[:, :, :, osl])  # [K,128,NCH,O_SH]
            g = gain[osl].reshape(OT, 128).T                 # [128, OT]
            b = bias[osl].reshape(OT, 128).T
            gb = np.ascontiguousarray(np.concatenate([g, b], axis=1)).astype(
                np.float32
            )
            in_maps.append({"hT": hT, "wT": w_core, "ab": ab, "gb": gb})
    return in_maps


def _assemble(results):
    outT = np.empty((H, B), np.float32)
    i = 0
    for rb in range(RB):
        for co in range(CO):
            r = results[i]["out"].reshape(O_SH, B_SH)       # [o, b]
            outT[co * O_SH : (co + 1) * O_SH, rb * B_SH : (rb + 1) * B_SH] = r
            i += 1
    return np.ascontiguousarray(outT.T)


def run(inputs, trace=False, tmpdir=None):
    nc = _build()
    in_maps = _prep_inputs(
        inputs["h"], inputs["W"], inputs["alpha"], inputs["beta"],
        inputs["gain"], inputs["bias"],
    )
    res = run_bass_kernel_spmd(
        nc, in_maps, core_ids=list(range(RB * CO)), trace=trace, tmpdir=tmpdir
    )
    return _assemble(res.results), res


def kernel(**inputs) -> np.ndarray:
    out, _ = run(inputs, trace=False)
    return out


if __name__ == "__main__":
    rng = np.random.default_rng(0)
    ins = {
        "t": np.zeros((1,), np.float32),
        "h": rng.standard_normal((B, H), dtype=np.float32),
        "W": (rng.standard_normal((H, H, K), dtype=np.float32) / np.sqrt(H)).astype(
            np.float32
        ),
        "alpha": rng.standard_normal((K,), dtype=np.float32),
        "beta": rng.standard_normal((K,), dtype=np.float32),
        "gain": np.ones((H,), np.float32),
        "bias": np.zeros((H,), np.float32),
    }
    out = kernel(**ins)
    s = np.tanh(ins["h"][:, :, None] * ins["alpha"] + ins["beta"])
    phi = np.einsum("bik,oik->bo", s, ins["W"]) / K
    exp = np.tanh(phi) * ins["gain"] + ins["bias"]
    err = np.linalg.norm(out - exp) / np.linalg.norm(exp)
    print("rel l2 err:", err)
